# revision 1
# baseline (speedup 1.0000x reference)
"""2-layer GAT + global mean pool + linear, sharded over 8 trn2 NeuronCores.

Strategy:
  - dst-sharded edge processing: core c owns dst nodes [c*B, (c+1)*B).
  - per core, dsts are degree-sorted and packed into a [P=128, NCH] grid of
    "chunks" (128 similar-degree dsts each). Each chunk's incoming edges sit
    at slots [p, k] (k < deg), gathered with one big indirect DMA per
    <=TMAX-column piece.
  - per-node tables Htab = [h(256) | a_s(4) | a_d(4)] rows; layer-1 table is
    computed redundantly on every core from x; layer-2 table is computed on
    own rows and AllGathered.
  - edge softmax: w = exp(lrelu(a_s[src] + a_d[dst])) with the segment-max
    dropped (logits are O(1); exp never overflows; alpha is identical).
  - aggregation: num = sum_k w*h_src, den = sum_k w per dst via DVE reduces;
    out = elu(num/den + bias).
  - global mean pool: per-chunk graph-onehot matmuls into PSUM, indirect
    scatter into a [NG+P, C] partial, AllReduce, scale by 1/cnt, final fc.
"""

import math
import sys

sys.path.insert(0, "/opt/trn_rl_repo")

import numpy as np

import concourse.bass as bass
import concourse.mybir as mybir
import concourse.tile as tile
from concourse import bacc
from concourse.masks import make_identity

P = 128
NEG_SLOPE = 0.2
DEBUG_L2 = False
USE_DMA_GATHER = True  # bulk gathers; needs single_packet=False
AS_PAD = -1.0e5  # a_s value for dummy rows -> exp(lrelu(...)) == 0


class Dims:
    def __init__(self, N=50000, F=128, C=256, H=4, OUT=64, NG=512, n_cores=8,
                 TMAX=32):
        self.N, self.F, self.C, self.H, self.OUT, self.NG = N, F, C, H, OUT, NG
        self.CH = C // H
        self.n_cores = n_cores
        assert N % n_cores == 0 and NG % P == 0
        self.B = N // n_cores              # dst nodes per core
        self.NCH = math.ceil(self.B / P)   # chunks per core
        self.GRID = self.NCH * P           # grid slots per core
        self.TMAX = TMAX
        self.NT1 = math.ceil((N + 4) / P)  # row tiles of layer-1 table
        self.NROW1 = self.NT1 * P
        self.R0 = 2                        # node n -> table row n+R0
        self.DLO1, self.PLO1 = 0, 1        # low-half dummy/pad rows
        self.DHI1, self.PHI1 = N + 2, N + 3
        self.DUMMY1 = N                    # pre-dual marker in off arrays
        self.SPLIT = 32768                 # int16 signed split point
        self.GRID_D = self.GRID + P        # grid + pad rows (dummy at GRID)
        self.ROW = C + 8                   # h | a_s | a_d
        self.ROWG = 384                    # gather-table row (768B bf16)
        self.NGP = NG + P                  # padded pool rows


class Plan:
    pass


def build_plan(edge_index, batch, dims: Dims):
    """All host-side layout decisions. Returns a Plan with per-core arrays."""
    d = dims
    src0 = np.asarray(edge_index[0], dtype=np.int64)
    dst0 = np.asarray(edge_index[1], dtype=np.int64)
    loop = np.arange(d.N, dtype=np.int64)
    src = np.concatenate([src0, loop]).astype(np.int32)
    dst = np.concatenate([dst0, loop]).astype(np.int32)
    batch = np.asarray(batch, dtype=np.int32)

    core_of = dst // d.B
    local = dst - core_of * d.B

    # per-core degree sort
    rank_of = np.empty((d.n_cores, d.B), dtype=np.int32)   # local id -> rank
    node_at = np.empty((d.n_cores, d.B), dtype=np.int32)   # rank -> local id
    K_g_core = np.zeros((d.n_cores, d.NCH), dtype=np.int32)
    for c in range(d.n_cores):
        deg = np.bincount(local[core_of == c], minlength=d.B)
        order = np.argsort(-deg, kind="stable").astype(np.int32)
        node_at[c] = order
        rank_of[c, order] = np.arange(d.B, dtype=np.int32)
        for g in range(d.NCH):
            r0 = g * P
            if r0 < d.B:
                K_g_core[c, g] = deg[order[r0]]

    K_g = np.maximum(K_g_core.max(axis=0), 1)
    col_start = np.zeros(d.NCH + 1, dtype=np.int64)
    col_start[1:] = np.cumsum(K_g)
    TOT = int(col_start[-1])

    # pieces: (g, kstart, ncols)
    pieces = []
    for g in range(d.NCH):
        k = 0
        while k < K_g[g]:
            n = min(d.TMAX, int(K_g[g]) - k)
            pieces.append((g, k, n))
            k += n

    # grid-flat row id of node n inside the concatenated layer-2 table
    n_core = np.arange(d.N, dtype=np.int64) // d.B
    n_local = np.arange(d.N, dtype=np.int64) % d.B
    n_rank = rank_of[n_core, n_local].astype(np.int64)
    grid_row = n_core * d.GRID_D + (n_rank % P) * d.NCH + (n_rank // P)

    # per-core arrays
    off1 = np.full((d.n_cores, P, TOT), d.DUMMY1, dtype=np.int32)
    off2 = np.empty((d.n_cores, P, TOT), dtype=np.int32)
    adidx1 = np.full((d.n_cores, P, d.NCH), 0, dtype=np.int32)
    gid_shift = np.full((d.n_cores, P, d.NCH), -1.0, dtype=np.float32)
    pool_row = np.zeros((d.n_cores, P, 1), dtype=np.int32)

    for c in range(d.n_cores):
        dummy2 = c * d.GRID_D + d.GRID  # dedicated pad row after the grid
        off2[c] = dummy2
        sel = np.nonzero(core_of == c)[0]
        s = src[sel]
        r = rank_of[c, local[sel]]
        o2 = np.argsort(r, kind="stable")
        s = s[o2]
        r = r[o2].astype(np.int64)
        n_e = len(r)
        first = np.ones(n_e, dtype=bool)
        first[1:] = r[1:] != r[:-1]
        starts = np.nonzero(first)[0]
        run_id = np.cumsum(first) - 1
        k = np.arange(n_e, dtype=np.int64) - starts[run_id]
        p = r % P
        g = r // P
        cols = col_start[g] + k
        assert (k < K_g[g]).all()
        off1[c, p, cols] = s
        off2[c, p, cols] = grid_row[s]

        # a_d gather indices + graph ids of the grid slots
        ranks = np.arange(d.B, dtype=np.int64)
        pp = ranks % P
        gg = ranks // P
        nodes = c * d.B + node_at[c].astype(np.int64)
        adidx1[c, pp, gg] = (nodes + d.R0).astype(np.int32)
        gmin = int(batch[c * d.B])
        gid_shift[c, pp, gg] = (batch[nodes] - gmin).astype(np.float32)
        span = int(batch[nodes].max()) - gmin + 1
        assert span <= P, f"graph span {span} > {P}"
        pool_row[c, :, 0] = gmin + np.arange(P)

    cnt = np.bincount(batch, minlength=d.NG).astype(np.float32)
    rcp_cnt = (1.0 / np.maximum(cnt, 1.0)).astype(np.float32)

    def wrap_dual(off, pad_mask, dlo, plo, dhi):
        # off: [n_cores, P, TOT] row ids; pad_mask True at true-pad slots.
        S = d.SPLIT
        lo = np.where(pad_mask, plo, np.where(off < S, off, dlo))
        hi = np.where(pad_mask | (off < S), dhi - S, off - S)
        assert (lo < S).all() and (lo >= 0).all() and (hi >= 0).all()
        outs = []
        for arr in (lo, hi):
            o = np.zeros((d.n_cores, P, 8 * TOT), dtype=np.int16)
            for (g, kst, ncols) in pieces:
                cs = int(col_start[g]) + kst
                for c in range(d.n_cores):
                    blk = arr[c][:, cs:cs + ncols]
                    lst = blk.T.reshape(-1).astype(np.int32).astype(np.int16)
                    w16 = lst.reshape(-1, 16).T
                    o[c][:, 8 * cs:8 * (cs + ncols)] = np.tile(w16, (8, 1))
            outs.append(o)
        return outs

    pl = Plan()
    pl.dims = d
    pl.K_g = K_g
    pl.col_start = col_start
    pl.TOT = TOT
    pl.pieces = pieces
    pl.off1, pl.off2 = off1, off2
    pad1 = off1 == d.DUMMY1
    off1r = off1.astype(np.int64) + d.R0   # node id -> table row
    pl.offg1a, pl.offg1b = wrap_dual(off1r, pad1, d.DLO1, d.PLO1, d.DHI1)
    # layer 2: every core's first pad row (c*GRID_D+GRID) is a dummy
    # (a_s = 0); the next one (+1) is a true pad (a_s = -1e5).
    dummies2 = [c * d.GRID_D + d.GRID for c in range(d.n_cores)]
    dlo2 = next(r for r in dummies2 if r < d.SPLIT)
    dhi2 = next(r for r in dummies2 if r >= d.SPLIT)
    plo2 = dlo2 + 1
    pad2 = np.zeros_like(pad1)
    for c in range(d.n_cores):
        pad2[c] = off2[c] == (c * d.GRID_D + d.GRID)
    pl.offg2a, pl.offg2b = wrap_dual(off2.astype(np.int64), pad2, dlo2, plo2,
                                     dhi2)
    pl.adidx1 = adidx1
    pl.gid_shift = gid_shift
    pl.pool_row = pool_row
    pl.rcp_cnt = rcp_cnt
    pl.grid_row = grid_row
    pl.n_pad_edges = d.n_cores * P * TOT - len(src)
    return pl


def _att_mat(att, d: Dims):
    """[H, CH] attention vector -> [C, H] block matrix so a = h @ A."""
    A = np.zeros((d.C, d.H), dtype=np.float32)
    for h in range(d.H):
        A[h * d.CH:(h + 1) * d.CH, h] = att[h]
    return A


def build_inputs(inputs, pl, np_dt):
    """Per-core in_maps (list of dicts) for the bass program."""
    d = pl.dims
    x = np.asarray(inputs["x"], np.float32)
    W1 = np.asarray(inputs["W1"], np.float32)
    W2 = np.asarray(inputs["W2"], np.float32)
    b1 = np.asarray(inputs["b1"], np.float32)
    b2 = np.asarray(inputs["b2"], np.float32)
    fc_w = np.asarray(inputs["fc_w"], np.float32)
    fc_b = np.asarray(inputs["fc_b"], np.float32)

    wcat1 = np.concatenate(
        [W1, W1 @ _att_mat(np.asarray(inputs["att_src1"], np.float32), d),
         W1 @ _att_mat(np.asarray(inputs["att_dst1"], np.float32), d)], axis=1)
    wcat2 = np.concatenate(
        [W2, W2 @ _att_mat(np.asarray(inputs["att_src2"], np.float32), d),
         W2 @ _att_mat(np.asarray(inputs["att_dst2"], np.float32), d)], axis=1)

    xT = np.zeros((d.F, d.NROW1), dtype=np_dt)
    xT[:, d.R0:d.R0 + d.N] = x.T.astype(np_dt)

    iota = np.tile(np.arange(P, dtype=np.float32), (P, 1))
    shared = {
        "xT": xT,
        "wcat1": wcat1.astype(np_dt),                      # [F, ROW]
        "wcat2": wcat2.astype(np_dt),                      # [C, ROW]
        "bias1": np.tile(b1.astype(np.float32), (P, 1)),   # [P, C]
        "bias2": np.tile(b2.astype(np.float32), (P, 1)),
        "fcw": fc_w.astype(np.float32),                    # [C, OUT]
        "fcb": np.tile(fc_b.astype(np.float32), (P, 1)),   # [P, OUT]
        "iota": iota,
        "rcp_cnt": pl.rcp_cnt.reshape(-1, P).T.copy(),     # [P, NG//P]
    }
    in_maps = []
    for c in range(d.n_cores):
        m = dict(shared)
        if USE_DMA_GATHER:
            m["off1"] = pl.offg1a[c]
            m["off1b"] = pl.offg1b[c]
            m["off2"] = pl.offg2a[c]
            m["off2b"] = pl.offg2b[c]
        else:
            m["off1"] = pl.off1[c]
            m["off2"] = pl.off2[c]
        m["adidx1"] = pl.adidx1[c]
        m["gid"] = pl.gid_shift[c]
        m["pool_row"] = pl.pool_row[c]
        in_maps.append(m)
    return in_maps


def build_program(pl, table_dt=mybir.dt.bfloat16):
    d = pl.dims
    f32 = mybir.dt.float32
    i32 = mybir.dt.int32
    DT = table_dt
    AF = mybir.ActivationFunctionType
    OP = mybir.AluOpType
    CT = d.C // P  # channel tiles (2)

    nc = bacc.Bacc("TRN2", target_bir_lowering=False, debug=False,
                   num_devices=d.n_cores)

    # ---- I/O ----
    xT = nc.dram_tensor("xT", [d.F, d.NROW1], DT, kind="ExternalInput")
    wcat1 = nc.dram_tensor("wcat1", [d.F, d.ROW], DT, kind="ExternalInput")
    wcat2 = nc.dram_tensor("wcat2", [d.C, d.ROW], DT, kind="ExternalInput")
    bias1 = nc.dram_tensor("bias1", [P, d.C], f32, kind="ExternalInput")
    bias2 = nc.dram_tensor("bias2", [P, d.C], f32, kind="ExternalInput")
    fcw = nc.dram_tensor("fcw", [d.C, d.OUT], f32, kind="ExternalInput")
    fcb = nc.dram_tensor("fcb", [P, d.OUT], f32, kind="ExternalInput")
    iota_t = nc.dram_tensor("iota", [P, P], f32, kind="ExternalInput")
    rcp_t = nc.dram_tensor("rcp_cnt", [P, d.NG // P], f32, kind="ExternalInput")
    i16 = mybir.dt.int16
    OFFW = 8 * pl.TOT if USE_DMA_GATHER else pl.TOT
    OFFDT = i16 if USE_DMA_GATHER else i32
    off1_t = nc.dram_tensor("off1", [P, OFFW], OFFDT, kind="ExternalInput")
    off2_t = nc.dram_tensor("off2", [P, OFFW], OFFDT, kind="ExternalInput")
    if USE_DMA_GATHER:
        off1b_t = nc.dram_tensor("off1b", [P, OFFW], i16, kind="ExternalInput")
        off2b_t = nc.dram_tensor("off2b", [P, OFFW], i16, kind="ExternalInput")
    adidx1_t = nc.dram_tensor("adidx1", [P, d.NCH], i32, kind="ExternalInput")
    gid_t = nc.dram_tensor("gid", [P, d.NCH], f32, kind="ExternalInput")
    pool_row_t = nc.dram_tensor("pool_row", [P, 1], i32, kind="ExternalInput")
    out_t = nc.dram_tensor("out", [d.NG, d.OUT], f32, kind="ExternalOutput")

    # ---- internal DRAM ----
    TW = d.ROWG if USE_DMA_GATHER else d.ROW
    htab1 = nc.dram_tensor("htab1", [d.NROW1, TW], DT, kind="Internal")
    l1out = nc.dram_tensor("l1out", [d.GRID, d.C], DT, kind="Internal")
    l2dbg = nc.dram_tensor("l2dbg", [d.GRID, d.C], f32, kind="Internal") \
        if DEBUG_L2 else None

    NGRID_ALL = d.GRID_D * d.n_cores

    with tile.TileContext(nc) as tc:
        with tc.tile_pool(name="const", bufs=1) as constp, \
             tc.tile_pool(name="work", bufs=3) as work, \
             tc.tile_pool(name="gath", bufs=2) as gath, \
             tc.tile_pool(name="small", bufs=3) as small, \
             tc.tile_pool(name="acc", bufs=2) as accp, \
             tc.tile_pool(name="psA", bufs=2, space="PSUM") as psA, \
             tc.tile_pool(name="psB", bufs=2, space="PSUM") as psB, \
             tc.tile_pool(name="psC", bufs=2, space="PSUM") as psC, \
             tc.tile_pool(name="pacc", bufs=1, space="PSUM") as paccp, \
             tc.tile_pool(name="dram", bufs=1, space="DRAM") as dram:

            # collectives need internal DRAM tiles
            htab2own = dram.tile([d.GRID_D, TW], DT)
            htab2all = dram.tile([NGRID_ALL, TW], DT, addr_space="Shared")
            poolpart = dram.tile([d.NGP, d.C], f32)
            poolsum = dram.tile([d.NGP, d.C], f32, addr_space="Shared")

            # ---- persistent SBUF constants ----
            wcat1_sb = constp.tile([d.F, d.ROW], DT, tag="wcat1")
            nc.sync.dma_start(out=wcat1_sb[:], in_=wcat1[:])
            wcat2_sb = constp.tile([P, CT, d.ROW], DT, tag="wcat2")
            nc.sync.dma_start(
                out=wcat2_sb[:],
                in_=wcat2[:].rearrange("(t p) r -> p t r", p=P))
            bias1_sb = constp.tile([P, d.C], f32, tag="bias1")
            nc.sync.dma_start(out=bias1_sb[:], in_=bias1[:])
            bias2_sb = constp.tile([P, d.C], f32, tag="bias2")
            nc.sync.dma_start(out=bias2_sb[:], in_=bias2[:])
            iota_sb = constp.tile([P, P], f32, tag="iota")
            nc.sync.dma_start(out=iota_sb[:], in_=iota_t[:])
            zeros_sb = constp.tile([P, d.C], f32, tag="zeros")
            nc.vector.memset(zeros_sb[:], 0.0)
            ident = constp.tile([P, P], DT, tag="ident")
            make_identity(nc, ident[:])
            ident32 = constp.tile([P, P], f32, tag="ident32")
            make_identity(nc, ident32[:])
            if not USE_DMA_GATHER:
                off1_sb = constp.tile([P, OFFW], OFFDT, tag="off1")
                nc.sync.dma_start(out=off1_sb[:], in_=off1_t[:])
                off2_sb = constp.tile([P, OFFW], OFFDT, tag="off2")
                nc.sync.dma_start(out=off2_sb[:], in_=off2_t[:])
            else:
                off1_sb = off2_sb = None
            adidx1_sb = constp.tile([P, d.NCH], i32, tag="adidx1")
            nc.sync.dma_start(out=adidx1_sb[:], in_=adidx1_t[:])
            gid_sb = constp.tile([P, d.NCH], f32, tag="gid")
            nc.sync.dma_start(out=gid_sb[:], in_=gid_t[:])
            pool_row_sb = constp.tile([P, 1], i32, tag="pool_row")
            nc.sync.dma_start(out=pool_row_sb[:], in_=pool_row_t[:])
            neg_sb = constp.tile([P, 8], DT, tag="neg")
            nc.vector.memset(neg_sb[:], AS_PAD)

            # =========== phase 1: Htab1 = [x @ Wcat1] for all nodes =========
            for nt in range(d.NT1):
                xt = work.tile([d.F, P], DT, tag="xt")
                nc.sync.dma_start(out=xt[:], in_=xT[:, nt * P:(nt + 1) * P])
                ps = psA.tile([P, d.ROW], f32, tag="mmps")
                nc.tensor.matmul(ps[:], lhsT=xt[:], rhs=wcat1_sb[:],
                                 start=True, stop=True)
                ht = work.tile([P, d.ROW], DT, tag="ht")
                nc.scalar.activation(ht[:], ps[:], AF.Copy)
                nc.sync.dma_start(out=htab1[nt * P:(nt + 1) * P, 0:d.ROW],
                                  in_=ht[:])
            # dummy row: a_s/a_d = AS_PAD
            nc.sync.dma_start(out=htab1[d.PLO1:d.PLO1 + 1, d.C:d.C + 8],
                              in_=neg_sb[0:1, :])
            nc.sync.dma_start(out=htab1[d.PHI1:d.PHI1 + 1, d.C:d.C + 8],
                              in_=neg_sb[0:1, :])
            if USE_DMA_GATHER:
                padz = constp.tile([P, d.ROWG - d.ROW], DT, tag="padz")
                nc.vector.memset(padz[:], 0.0)
                nc.sync.dma_start(
                    out=htab1[d.DUMMY1:d.DUMMY1 + 1, d.ROW:d.ROWG],
                    in_=padz[0:1, :])

            # ============ edge phase helper ============
            def edge_layer(htab_ap, off_sb, adg_sb, out_cb, offp=None):
                """Process all chunks; out_cb(g, accn[P,C] f32, accd[P,H])."""
                accn = accd = None
                for (g, kst, ncols) in pl.pieces:
                    piece_first = kst == 0
                    piece_last = kst + ncols == pl.K_g[g]
                    cs = int(pl.col_start[g]) + kst
                    if USE_DMA_GATHER:
                        lo_ap, hi_ap, offa_t, offb_t = offp
                        nidx = P * ncols
                        ita = small.tile([P, 8 * d.TMAX], i16, tag="ita")
                        nc.sync.dma_start(
                            out=ita[:, 0:8 * ncols],
                            in_=offa_t[:, 8 * cs:8 * (cs + ncols)])
                        itb = small.tile([P, 8 * d.TMAX], i16, tag="itb")
                        nc.sync.dma_start(
                            out=itb[:, 0:8 * ncols],
                            in_=offb_t[:, 8 * cs:8 * (cs + ncols)])
                        gt = gath.tile([P, d.TMAX, d.ROWG], DT, tag="gt")
                        nc.gpsimd.dma_gather(
                            out_ap=gt[:, 0:ncols, :], in_ap=lo_ap,
                            idxs_ap=ita[:, 0:8 * ncols],
                            num_idxs=nidx, num_idxs_reg=nidx,
                            elem_size=d.ROWG, single_packet=False)
                        gtb = gath.tile([P, d.TMAX, d.ROWG], DT, tag="gtb")
                        nc.gpsimd.dma_gather(
                            out_ap=gtb[:, 0:ncols, :], in_ap=hi_ap,
                            idxs_ap=itb[:, 0:8 * ncols],
                            num_idxs=nidx, num_idxs_reg=nidx,
                            elem_size=d.ROWG, single_packet=False)
                        # merge halves (dummy rows are additive identities)
                        nc.vector.tensor_tensor(
                            out=gt[:, 0:ncols, 0:d.ROW],
                            in0=gt[:, 0:ncols, 0:d.ROW],
                            in1=gtb[:, 0:ncols, 0:d.ROW],
                            op=OP.add)
                    else:
                        gt = gath.tile([P, d.TMAX, d.ROW], DT, tag="gt")
                        for kc in range(ncols):
                            nc.gpsimd.indirect_dma_start(
                                out=gt[:, kc, :],
                                out_offset=None,
                                in_=htab_ap,
                                in_offset=bass.IndirectOffsetOnAxis(
                                    ap=off_sb[:, cs + kc:cs + kc + 1], axis=0),
                            )
                    # logits = a_s[src] + a_d[dst]
                    lg = small.tile([P, d.TMAX, d.H], f32, tag="lg")
                    nc.vector.tensor_tensor(
                        out=lg[:, 0:ncols, :],
                        in0=gt[:, 0:ncols, d.C:d.C + d.H],
                        in1=adg_sb[:, g:g + 1, :].to_broadcast(
                            (P, ncols, d.H)),
                        op=OP.add)
                    # exp(lrelu(x)) == max(exp(x), exp(0.2*x))
                    wt = small.tile([P, d.TMAX, d.H], DT, tag="wt")
                    nc.scalar.activation(wt[:, 0:ncols, :], lg[:, 0:ncols, :],
                                         AF.Exp)
                    wb = small.tile([P, d.TMAX, d.H], DT, tag="wb")
                    nc.scalar.activation(wb[:, 0:ncols, :], lg[:, 0:ncols, :],
                                         AF.Exp, scale=NEG_SLOPE)
                    nc.vector.tensor_tensor(out=wt[:, 0:ncols, :],
                                            in0=wt[:, 0:ncols, :],
                                            in1=wb[:, 0:ncols, :], op=OP.max)
                    if piece_first:
                        accn = accp.tile([P, d.C], f32, tag="accn")
                        accd = accp.tile([P, d.H], f32, tag="accd")
                        nout, dout = accn, accd
                    else:
                        nout = accp.tile([P, d.C], f32, tag="npart")
                        dout = small.tile([P, d.H], f32, tag="dpart")
                    # den partial
                    nc.vector.tensor_reduce(
                        out=dout[:],
                        in_=wt[:, 0:ncols, :].rearrange("p k h -> p h k"),
                        axis=mybir.AxisListType.X, op=OP.add)
                    # messages: h *= w (broadcast over channels)
                    hview = gt[:, 0:ncols, 0:d.C].rearrange(
                        "p k (h ch) -> p k h ch", h=d.H)
                    nc.vector.tensor_tensor(
                        out=hview, in0=hview,
                        in1=wt[:, 0:ncols, :].to_broadcast(
                            (P, ncols, d.H, d.CH)),
                        op=OP.mult)
                    nc.vector.tensor_reduce(
                        out=nout[:],
                        in_=gt[:, 0:ncols, 0:d.C].rearrange("p k c -> p c k"),
                        axis=mybir.AxisListType.X, op=OP.add)
                    if not piece_first:
                        nc.vector.tensor_add(accn[:], accn[:], nout[:])
                        nc.vector.tensor_add(accd[:], accd[:], dout[:])
                    if piece_last:
                        out_cb(g, accn, accd)

            def epilogue(accn, accd, bias_sb, out_tile):
                """out_tile = elu(num/den + bias)"""
                nc.vector.tensor_scalar_max(accd[:], accd[:], 1e-20)
                rcp = small.tile([P, d.H], f32, tag="rcp")
                nc.vector.reciprocal(rcp[:], accd[:])
                x_ = small.tile([P, d.C], f32, tag="x_")
                nc.vector.tensor_tensor(
                    out=x_[:].rearrange("p (h ch) -> p h ch", h=d.H),
                    in0=accn[:].rearrange("p (h ch) -> p h ch", h=d.H),
                    in1=rcp[:].to_broadcast((P, d.H, d.CH)),
                    op=OP.mult)
                nc.vector.tensor_add(x_[:], x_[:], bias_sb[:])
                # elu = max(x,0) + min(exp(x)-1, 0)
                ex = small.tile([P, d.C], f32, tag="ex")
                nc.scalar.activation(ex[:], x_[:], AF.Exp)
                nc.vector.scalar_tensor_tensor(
                    out=ex[:], in0=ex[:], scalar=-1.0, in1=zeros_sb[:],
                    op0=OP.add, op1=OP.min)
                nc.vector.tensor_scalar_max(x_[:], x_[:], 0.0)
                nc.vector.tensor_tensor(out=out_tile[:], in0=x_[:], in1=ex[:],
                                        op=OP.add)

            # =================== layer 1 ===================
            adg1 = constp.tile([P, d.NCH, d.H], DT, tag="adg1")
            for gg_ in range(d.NCH):
                nc.gpsimd.indirect_dma_start(
                    out=adg1[:, gg_, :], out_offset=None, in_=htab1[:, :],
                    in_offset=bass.IndirectOffsetOnAxis(
                        ap=adidx1_sb[:, gg_:gg_ + 1], axis=0),
                    element_offset=d.C + d.H)

            def l1_out(g, accn, accd):
                et = work.tile([P, d.C], DT, tag="et1")
                epilogue(accn, accd, bias1_sb, et)
                nc.sync.dma_start(
                    out=l1out[:].rearrange("(p n) c -> p n c", p=P)[:, g, :],
                    in_=et[:])

            l1p = ((htab1[0:d.SPLIT, :], htab1[d.SPLIT:d.NROW1, :],
                    off1_t, off1b_t) if USE_DMA_GATHER else None)
            edge_layer(htab1[:, :], off1_sb, adg1, l1_out, offp=l1p)

            # ======== layer-2 table: htab2own = elu1 @ Wcat2, AllGather ======
            for g in range(d.NCH):
                el = work.tile([P, d.C], DT, tag="el")
                nc.sync.dma_start(
                    out=el[:],
                    in_=l1out[:].rearrange("(p n) c -> p n c", p=P)[:, g, :])
                elT = work.tile([P, CT, P], DT, tag="elT")
                for it in range(CT):
                    tp = psB.tile([P, P], DT, tag="tp")
                    nc.tensor.transpose(tp[:], el[:, it * P:(it + 1) * P],
                                        ident[:])
                    nc.scalar.activation(elT[:, it, :], tp[:], AF.Copy)
                ps2 = psA.tile([P, d.ROW], f32, tag="mmps")
                for it in range(CT):
                    nc.tensor.matmul(ps2[:], lhsT=elT[:, it, :],
                                     rhs=wcat2_sb[:, it, :],
                                     start=(it == 0), stop=(it == CT - 1))
                h2t = work.tile([P, d.ROW], DT, tag="ht")
                nc.scalar.activation(h2t[:], ps2[:], AF.Copy)
                nc.sync.dma_start(
                    out=htab2own[0:d.GRID, 0:d.ROW].rearrange(
                        "(p n) r -> p n r", p=P)[:, g, :],
                    in_=h2t[:])
            # dedicated dummy pad rows [GRID, GRID_D): h = 0, a_s/a_d = AS_PAD
            drow = work.tile([P, TW], DT, tag="drow")
            nc.vector.memset(drow[:, 0:d.C], 0.0)
            nc.vector.memset(drow[:, d.C:TW], AS_PAD)
            nc.vector.memset(drow[0:1, d.C:TW], 0.0)  # dummy row: a_s = 0
            nc.sync.dma_start(out=htab2own[d.GRID:d.GRID_D, :], in_=drow[:])
            nc.gpsimd.collective_compute(
                "AllGather", OP.bypass,
                replica_groups=[list(range(d.n_cores))],
                ins=[htab2own.opt()], outs=[htab2all.opt()])

            # =================== layer 2 + pooling ===================
            adg2 = constp.tile([P, d.NCH, d.H], DT, tag="adg2")
            nc.sync.dma_start(
                out=adg2[:],
                in_=htab2own[0:d.GRID, 0:d.ROW].rearrange(
                    "(p n) r -> p n r", p=P)[:, :, d.C + d.H:d.C + 2 * d.H])

            pool_ps = paccp.tile([P, d.C], f32, tag="poolps")

            def l2_out(g, accn, accd):
                et = work.tile([P, d.C], f32, tag="et2")
                epilogue(accn, accd, bias2_sb, et)
                if l2dbg is not None:
                    nc.sync.dma_start(
                        out=l2dbg[:].rearrange("(p n) c -> p n c", p=P)[:, g, :],
                        in_=et[:])
                oh = work.tile([P, P], f32, tag="oh")
                nc.vector.tensor_tensor(
                    out=oh[:],
                    in0=gid_sb[:, g:g + 1].to_broadcast((P, P)),
                    in1=iota_sb[:], op=OP.is_equal)
                nc.tensor.matmul(pool_ps[:], lhsT=oh[:], rhs=et[:],
                                 start=(g == 0), stop=(g == d.NCH - 1))

            l2p = ((htab2all[0:d.SPLIT, :],
                    htab2all[d.SPLIT:NGRID_ALL, :],
                    off2_t, off2b_t) if USE_DMA_GATHER else None)
            edge_layer(htab2all[:, :], off2_sb, adg2, l2_out, offp=l2p)

            # pool partial -> DRAM [NGP, C] zeroed, scatter own window
            zt = work.tile([P, d.C], f32, tag="zt")
            nc.vector.memset(zt[:], 0.0)
            for t in range(d.NGP // P):
                nc.sync.dma_start(out=poolpart[t * P:(t + 1) * P, :], in_=zt[:])
            pool_sb = work.tile([P, d.C], f32, tag="poolsb")
            nc.vector.tensor_copy(pool_sb[:], pool_ps[:])
            nc.gpsimd.indirect_dma_start(
                out=poolpart[:, :],
                out_offset=bass.IndirectOffsetOnAxis(ap=pool_row_sb[:, 0:1],
                                                     axis=0),
                in_=pool_sb[:], in_offset=None)
            nc.gpsimd.collective_compute(
                "AllReduce", OP.add,
                replica_groups=[list(range(d.n_cores))],
                ins=[poolpart.opt()], outs=[poolsum.opt()])

            # mean + fc
            rcp_sb = constp.tile([P, d.NG // P], f32, tag="rcp_cnt")
            nc.sync.dma_start(out=rcp_sb[:], in_=rcp_t[:])
            fcw_sb = constp.tile([P, CT, d.OUT], f32, tag="fcw")
            nc.sync.dma_start(
                out=fcw_sb[:], in_=fcw[:].rearrange("(t p) o -> p t o", p=P))
            fcb_sb = constp.tile([P, d.OUT], f32, tag="fcb")
            nc.sync.dma_start(out=fcb_sb[:], in_=fcb[:])
            for t in range(d.NG // P):
                pm = work.tile([P, d.C], f32, tag="pm")
                nc.sync.dma_start(out=pm[:], in_=poolsum[t * P:(t + 1) * P, :])
                nc.vector.tensor_scalar(
                    out=pm[:], in0=pm[:], scalar1=rcp_sb[:, t:t + 1],
                    scalar2=None, op0=OP.mult)
                pmT = work.tile([P, CT, P], f32, tag="pmT")
                for it in range(CT):
                    tp = psB.tile([P, P], f32, tag="tp")
                    nc.tensor.transpose(tp[:], pm[:, it * P:(it + 1) * P],
                                        ident32[:])
                    nc.vector.tensor_copy(pmT[:, it, :], tp[:])
                ops = psC.tile([P, d.OUT], f32, tag="ops")
                for it in range(CT):
                    nc.tensor.matmul(ops[:], lhsT=pmT[:, it, :],
                                     rhs=fcw_sb[:, it, :],
                                     start=(it == 0), stop=(it == CT - 1))
                ot = work.tile([P, d.OUT], f32, tag="ot")
                nc.vector.tensor_add(ot[:], ops[:], fcb_sb[:])
                nc.sync.dma_start(out=out_t[t * P:(t + 1) * P, :], in_=ot[:])

    nc.compile()
    return nc


def np_dt_of(table_dt):
    import ml_dtypes
    return {mybir.dt.bfloat16: ml_dtypes.bfloat16,
            mybir.dt.float32: np.float32}[table_dt]


def run_kernel_full(inputs, table_dt=mybir.dt.bfloat16, dims=None, sim=False,
                    nc=None, pl=None):
    """Full pipeline: plan, build, run on 8 cores, return [NG, OUT] f32."""
    d = dims or Dims()
    if pl is None:
        pl = build_plan(np.asarray(inputs["edge_index"]),
                        np.asarray(inputs["batch"]), d)
    in_maps = build_inputs(inputs, pl, np_dt_of(table_dt))
    if nc is None:
        nc = build_program(pl, table_dt)
    if sim:
        from concourse.bass_interp import MultiCoreSim
        ms = MultiCoreSim(nc, num_cores=d.n_cores, trace=False,
                          require_finite=False, require_nnan=False)
        for c, core in enumerate(ms.cores.values()):
            for k, v in in_maps[c].items():
                core.tensor(k)[:] = v
        ms.simulate(check_with_hw=False)
        return np.asarray(list(ms.cores.values())[0].tensor("out"))
    from concourse.bass_utils import run_bass_kernel_spmd
    res = run_bass_kernel_spmd(nc, in_maps, core_ids=list(range(d.n_cores)))
    return res.results[0]["out"]

TABLE_DT = mybir.dt.bfloat16


# ======================= harness entry point =======================

_CACHE = {}


def kernel(**inputs):
    """Full (unsharded) inputs -> full [512, 64] float32 output.

    Shards nodes/edges across 8 NeuronCores internally (dst-block
    partitioning of edge_index per the degree-sorted grid layout),
    compiles the Bass program for this graph, and runs it SPMD on
    cores 0-7 via run_bass_kernel_spmd.
    """
    from concourse.bass_utils import run_bass_kernel_spmd

    d = Dims()  # hardcoded problem dims: N=50000, F=128, C=256, NG=512
    ei = np.asarray(inputs["edge_index"])
    bt = np.asarray(inputs["batch"])
    key = (ei.tobytes(), bt.tobytes())
    if key in _CACHE:
        pl, nc = _CACHE[key]
    else:
        pl = build_plan(ei, bt, d)
        nc = build_program(pl, TABLE_DT)
        _CACHE[key] = (pl, nc)
    in_maps = build_inputs(inputs, pl, np_dt_of(TABLE_DT))
    res = run_bass_kernel_spmd(nc, in_maps, core_ids=list(range(d.n_cores)))
    return np.asarray(res.results[0]["out"], dtype=np.float32)


if __name__ == "__main__":
    rng = np.random.default_rng(0)
    print("kernel.py self-check: building plan only")



# revision 9
# speedup vs baseline: 3.1144x; 3.1144x over previous
"""2-layer GAT + global mean pool + linear, sharded over 8 trn2 NeuronCores.

v2 strategy (vs v1 which dual-gathered 768B rows for both layers):
  - One dst grid per core: core c owns dsts [c*B, (c+1)*B), sorted by
    max(lo_deg, hi_deg) (lo/hi = src node < / >= N/2), packed [P=128, NCH]
    chunks of similar degree.
  - Layer 1 needs NO device gathers: the edge structure is static, so the
    host pre-expands x[src] per edge slot (xeT, F-major) and the device
    computes per-edge rows h|a_s|a_d = xe @ Wcat1 with PE matmuls (PE was
    idle). Pad slots are masked via an additive -1e5 on the logits.
  - Layer 2 table htab2own (rows = elu(l1) @ Wcat2 in grid-rank order) is
    AllGathered; per-edge rows are dma_gathered ONCE per slot: chunk columns
    split into a lo-piece set (table rows < 4*GRID_D, int16-addressable) and
    a hi-piece set (rows rebased by -4*GRID_D). No dual gather, no merge.
  - Edge softmax drops the segment-max (logits O(1); exp never overflows).
  - num/den accumulate per chunk across lo+hi pieces; epilogue
    elu(num/den + bias); pooling via per-chunk graph-onehot matmuls,
    AllReduce, final fc.
"""

import math
import sys

sys.path.insert(0, "/opt/trn_rl_repo")

import numpy as np

import concourse.bass as bass
import concourse.mybir as mybir
import concourse.tile as tile
from concourse import bacc
from concourse.masks import make_identity

P = 128
NEG_SLOPE = 0.2
AS_PAD = -1.0e5


class Dims:
    def __init__(self, N=50000, F=128, C=256, H=4, OUT=64, NG=512, n_cores=8,
                 TMAX=32):
        self.N, self.F, self.C, self.H, self.OUT, self.NG = N, F, C, H, OUT, NG
        self.CH = C // H
        self.n_cores = n_cores
        assert N % n_cores == 0 and NG % P == 0
        self.B = N // n_cores              # dst nodes per core
        self.NCH = math.ceil(self.B / P)   # chunks per core
        self.GRID = self.NCH * P           # real grid rows per core
        self.GRID_D = self.GRID + P        # + pad rows (AS_PAD rows)
        self.TMAX = TMAX
        self.ROW = C + 8                   # h | a_s | a_d
        self.ROWG = 384                    # padded row for dma_gather (768B)
        self.HALF = N // 2                 # lo/hi src split (by node id)
        self.LO_ROWS = (n_cores // 2) * self.GRID_D
        self.PADROW = self.GRID            # rebased pad-row idx (both halves)
        self.NGP = NG + P                  # padded pool rows


class Plan:
    pass


def _fill_slots(sel_rank, col_start, K_g, out, val):
    """Place per-edge values (keyed by dst rank) into [P, TOT] grid slots."""
    o = np.argsort(sel_rank, kind="stable")
    r = sel_rank[o].astype(np.int64)
    v = val[o]
    n_e = len(r)
    if n_e == 0:
        return
    first = np.ones(n_e, dtype=bool)
    first[1:] = r[1:] != r[:-1]
    starts = np.nonzero(first)[0]
    run_id = np.cumsum(first) - 1
    k = np.arange(n_e, dtype=np.int64) - starts[run_id]
    p = r % P
    g = r // P
    assert (k < K_g[g]).all()
    cols = col_start[g] + k
    out[p, cols] = v


def _pieces_of(K_g, TMAX):
    col_start = np.zeros(len(K_g) + 1, dtype=np.int64)
    col_start[1:] = np.cumsum(K_g)
    pieces = []
    for g in range(len(K_g)):
        k = 0
        while k < K_g[g]:
            n = min(TMAX, int(K_g[g]) - k)
            pieces.append((g, k, n))
            k += n
    return col_start, pieces


def _wrap16(arr, pieces, col_start, n_cores):
    """[n_cores, P, TOT] int32 -> [n_cores, P, 8*TOT] int16 gather-idx wrap."""
    nc_, _, TOT = arr.shape
    out = np.zeros((nc_, P, 8 * TOT), dtype=np.int16)
    for (g, kst, ncols) in pieces:
        cs = int(col_start[g]) + kst
        for c in range(n_cores):
            blk = arr[c][:, cs:cs + ncols]
            lst = blk.T.reshape(-1).astype(np.int32)
            assert (lst >= 0).all() and (lst < 32768).all()
            w16 = lst.astype(np.int16).reshape(-1, 16).T
            out[c][:, 8 * cs:8 * (cs + ncols)] = np.tile(w16, (8, 1))
    return out


def build_plan(edge_index, batch, dims: Dims):
    d = dims
    src0 = np.asarray(edge_index[0], dtype=np.int64)
    dst0 = np.asarray(edge_index[1], dtype=np.int64)
    loop = np.arange(d.N, dtype=np.int64)
    src = np.concatenate([src0, loop]).astype(np.int64)
    dst = np.concatenate([dst0, loop]).astype(np.int64)
    batch = np.asarray(batch, dtype=np.int32)

    core_of = dst // d.B
    local = dst - core_of * d.B
    is_hi = src >= d.HALF

    rank_of = np.empty((d.n_cores, d.B), dtype=np.int32)
    node_at = np.full((d.n_cores, d.B), -1, dtype=np.int32)  # rank -> local
    K1c = np.zeros((d.n_cores, d.NCH), dtype=np.int32)
    Kloc = np.zeros((d.n_cores, d.NCH), dtype=np.int32)
    Khic = np.zeros((d.n_cores, d.NCH), dtype=np.int32)
    for c in range(d.n_cores):
        m = core_of == c
        dlo = np.bincount(local[m & ~is_hi], minlength=d.B)
        dhi = np.bincount(local[m & is_hi], minlength=d.B)
        dtot = dlo + dhi
        key = np.maximum(dlo, dhi).astype(np.int64) * 1000 + dtot
        order = np.argsort(-key, kind="stable").astype(np.int32)
        node_at[c] = order
        rank_of[c, order] = np.arange(d.B, dtype=np.int32)
        for g in range(d.NCH):
            r0 = g * P
            if r0 < d.B:
                blk = order[r0:min(r0 + P, d.B)]
                K1c[c, g] = dtot[blk].max()
                Kloc[c, g] = dlo[blk].max()
                Khic[c, g] = dhi[blk].max()

    K1 = np.maximum(K1c.max(axis=0), 1)
    Klo = Kloc.max(axis=0)
    Khi = Khic.max(axis=0)
    cs1, pieces1 = _pieces_of(K1, d.TMAX)
    cslo, pieces_lo = _pieces_of(Klo, d.TMAX)
    cshi, pieces_hi = _pieces_of(Khi, d.TMAX)
    TOT1, TOTlo, TOThi = int(cs1[-1]), int(cslo[-1]), int(cshi[-1])

    # layer-2 chunk-interleaved piece list: per chunk, lo pieces then hi
    by_chunk = {g: [] for g in range(d.NCH)}
    for pc in pieces_lo:
        by_chunk[pc[0]].append(("lo",) + pc)
    for pc in pieces_hi:
        by_chunk[pc[0]].append(("hi",) + pc)
    pieces2 = []
    for g in range(d.NCH):
        lst = by_chunk[g]
        assert lst, f"chunk {g} has no layer-2 pieces"
        for i, pc in enumerate(lst):
            pieces2.append(pc + (i == 0, i == len(lst) - 1))

    # global table row of each node (layer-2 gather target)
    n_core = np.arange(d.N, dtype=np.int64) // d.B
    n_local = np.arange(d.N, dtype=np.int64) % d.B
    n_rank = rank_of[n_core, n_local].astype(np.int64)
    grid_row = n_core * d.GRID_D + n_rank

    srcs1 = np.zeros((d.n_cores, P, TOT1), dtype=np.int32)
    mask1 = np.full((d.n_cores, P, TOT1), AS_PAD, dtype=np.float32)
    off_lo = np.full((d.n_cores, P, max(TOTlo, 1)), d.PADROW, dtype=np.int32)
    off_hi = np.full((d.n_cores, P, max(TOThi, 1)), d.PADROW, dtype=np.int32)
    gid_shift = np.full((d.n_cores, P, d.NCH), -1.0, dtype=np.float32)
    pool_row = np.zeros((d.n_cores, P, 1), dtype=np.int32)
    xd_nodes = np.zeros((d.n_cores, d.GRID), dtype=np.int64)

    for c in range(d.n_cores):
        m = np.nonzero(core_of == c)[0]
        r_all = rank_of[c, local[m]]
        _fill_slots(r_all, cs1, K1, srcs1[c], src[m].astype(np.int32))
        _fill_slots(r_all, cs1, K1, mask1[c],
                    np.zeros(len(m), dtype=np.float32))
        mlo = m[~is_hi[m]]
        _fill_slots(rank_of[c, local[mlo]], cslo, Klo, off_lo[c],
                    grid_row[src[mlo]].astype(np.int32))
        mhi = m[is_hi[m]]
        _fill_slots(rank_of[c, local[mhi]], cshi, Khi, off_hi[c],
                    (grid_row[src[mhi]] - d.LO_ROWS).astype(np.int32))

        nodes = c * d.B + node_at[c].astype(np.int64)
        xd_nodes[c, :d.B] = nodes
        xd_nodes[c, d.B:] = nodes[0]
        ranks = np.arange(d.B, dtype=np.int64)
        pp, gg = ranks % P, ranks // P
        gmin = int(batch[c * d.B])
        gid_shift[c, pp, gg] = (batch[nodes] - gmin).astype(np.float32)
        span = int(batch[nodes].max()) - gmin + 1
        assert span <= P, f"graph span {span} > {P}"
        pool_row[c, :, 0] = gmin + np.arange(P)

    cnt = np.bincount(batch, minlength=d.NG).astype(np.float32)
    rcp_cnt = (1.0 / np.maximum(cnt, 1.0)).astype(np.float32)

    pl = Plan()
    pl.dims = d
    pl.K1, pl.Klo, pl.Khi = K1, Klo, Khi
    pl.cs1, pl.cslo, pl.cshi = cs1, cslo, cshi
    pl.pieces1, pl.pieces2 = pieces1, pieces2
    pl.TOT1, pl.TOTlo, pl.TOThi = TOT1, TOTlo, TOThi
    pl.srcs1, pl.mask1 = srcs1, mask1
    pl.offg_lo = _wrap16(off_lo, pieces_lo, cslo, d.n_cores)
    pl.offg_hi = _wrap16(off_hi, pieces_hi, cshi, d.n_cores)
    pl.gid_shift, pl.pool_row, pl.rcp_cnt = gid_shift, pool_row, rcp_cnt
    pl.xd_nodes = xd_nodes
    return pl


def _att_mat(att, d: Dims):
    A = np.zeros((d.C, d.H), dtype=np.float32)
    for h in range(d.H):
        A[h * d.CH:(h + 1) * d.CH, h] = att[h]
    return A


def build_inputs(inputs, pl, np_dt):
    d = pl.dims
    x = np.asarray(inputs["x"], np.float32)
    W1 = np.asarray(inputs["W1"], np.float32)
    W2 = np.asarray(inputs["W2"], np.float32)
    b1 = np.asarray(inputs["b1"], np.float32)
    b2 = np.asarray(inputs["b2"], np.float32)
    fc_w = np.asarray(inputs["fc_w"], np.float32)
    fc_b = np.asarray(inputs["fc_b"], np.float32)

    wcat1 = np.concatenate(
        [W1, W1 @ _att_mat(np.asarray(inputs["att_src1"], np.float32), d),
         W1 @ _att_mat(np.asarray(inputs["att_dst1"], np.float32), d)], axis=1)
    wcat2 = np.concatenate(
        [W2, W2 @ _att_mat(np.asarray(inputs["att_src2"], np.float32), d),
         W2 @ _att_mat(np.asarray(inputs["att_dst2"], np.float32), d)],
        axis=1)
    wad1 = W1 @ _att_mat(np.asarray(inputs["att_dst1"], np.float32), d)

    xTb = np.ascontiguousarray(x.T).astype(np_dt)      # [F, N]
    iota = np.tile(np.arange(P, dtype=np.float32), (P, 1))
    shared = {
        "wcat1": wcat1.astype(np_dt),                  # [F, ROW]
        "wad1": wad1.astype(np_dt),                    # [F, H]
        "wcat2": wcat2.astype(np_dt),                  # [C, ROWG]
        "bias1": np.tile(b1.astype(np.float32), (P, 1)),
        "bias2": np.tile(b2.astype(np.float32), (P, 1)),
        "fcw": fc_w.astype(np.float32),
        "fcb": np.tile(fc_b.astype(np.float32), (P, 1)),
        "iota": iota,
        "rcp_cnt": pl.rcp_cnt.reshape(-1, P).T.copy(),
    }
    in_maps = []
    for c in range(d.n_cores):
        m = dict(shared)
        srcs_flat = pl.srcs1[c].T.reshape(-1)          # col-major slot order
        m["xeT"] = np.ascontiguousarray(xTb[:, srcs_flat])
        m["xdT"] = np.ascontiguousarray(xTb[:, pl.xd_nodes[c]])
        m["mask1"] = pl.mask1[c]
        m["off_lo"] = pl.offg_lo[c]
        m["off_hi"] = pl.offg_hi[c]
        m["gid"] = pl.gid_shift[c]
        m["pool_row"] = pl.pool_row[c]
        in_maps.append(m)
    return in_maps


def build_program(pl, table_dt=mybir.dt.bfloat16):
    d = pl.dims
    f32 = mybir.dt.float32
    i32 = mybir.dt.int32
    i16 = mybir.dt.int16
    DT = table_dt
    AF = mybir.ActivationFunctionType
    OP = mybir.AluOpType
    CT = d.C // P  # 2

    nc = bacc.Bacc("TRN2", target_bir_lowering=False, debug=False,
                   num_devices=d.n_cores)

    # ---- I/O ----
    xeT_t = nc.dram_tensor("xeT", [d.F, pl.TOT1 * P], DT, kind="ExternalInput")
    xdT_t = nc.dram_tensor("xdT", [d.F, d.GRID], DT, kind="ExternalInput")
    mask1_t = nc.dram_tensor("mask1", [P, pl.TOT1], f32, kind="ExternalInput")
    off_lo_t = nc.dram_tensor("off_lo", [P, 8 * max(pl.TOTlo, 1)], i16,
                              kind="ExternalInput")
    off_hi_t = nc.dram_tensor("off_hi", [P, 8 * max(pl.TOThi, 1)], i16,
                              kind="ExternalInput")
    wcat1_t = nc.dram_tensor("wcat1", [d.F, d.ROW], DT, kind="ExternalInput")
    wad1_t = nc.dram_tensor("wad1", [d.F, d.H], DT, kind="ExternalInput")
    wcat2_t = nc.dram_tensor("wcat2", [d.C, d.ROW], DT, kind="ExternalInput")
    bias1_t = nc.dram_tensor("bias1", [P, d.C], f32, kind="ExternalInput")
    bias2_t = nc.dram_tensor("bias2", [P, d.C], f32, kind="ExternalInput")
    fcw_t = nc.dram_tensor("fcw", [d.C, d.OUT], f32, kind="ExternalInput")
    fcb_t = nc.dram_tensor("fcb", [P, d.OUT], f32, kind="ExternalInput")
    iota_t = nc.dram_tensor("iota", [P, P], f32, kind="ExternalInput")
    rcp_t = nc.dram_tensor("rcp_cnt", [P, d.NG // P], f32,
                           kind="ExternalInput")
    gid_t = nc.dram_tensor("gid", [P, d.NCH], f32, kind="ExternalInput")
    pool_row_t = nc.dram_tensor("pool_row", [P, 1], i32, kind="ExternalInput")
    out_t = nc.dram_tensor("out", [d.NG, d.OUT], f32, kind="ExternalOutput")

    NGRID_ALL = d.GRID_D * d.n_cores

    with tile.TileContext(nc) as tc:
        with tc.tile_pool(name="const", bufs=1) as constp, \
             tc.tile_pool(name="xp", bufs=3) as xp, \
             tc.tile_pool(name="work", bufs=2) as work, \
             tc.tile_pool(name="gath", bufs=2) as gath, \
             tc.tile_pool(name="small", bufs=3) as small, \
             tc.tile_pool(name="acc", bufs=2) as accp, \
             tc.tile_pool(name="psA", bufs=3, space="PSUM") as psA, \
             tc.tile_pool(name="psB", bufs=2, space="PSUM") as psB, \
             tc.tile_pool(name="psC", bufs=1, space="PSUM") as psC, \
             tc.tile_pool(name="pacc", bufs=1, space="PSUM") as paccp, \
             tc.tile_pool(name="dram", bufs=1, space="DRAM") as dram:

            htab2own = dram.tile([d.GRID_D, d.ROWG], DT)
            htab2all = dram.tile([NGRID_ALL, d.ROWG], DT, addr_space="Shared")
            poolpart = dram.tile([d.NGP, d.C], f32)
            poolsum = dram.tile([d.NGP, d.C], f32, addr_space="Shared")

            # ---- persistent SBUF constants ----
            wcat1_sb = constp.tile([d.F, d.ROW], DT, tag="wcat1")
            nc.sync.dma_start(out=wcat1_sb[:], in_=wcat1_t[:])
            wad1_sb = constp.tile([d.F, d.H], DT, tag="wad1")
            nc.sync.dma_start(out=wad1_sb[:], in_=wad1_t[:])
            wcat2_sb = constp.tile([P, CT, d.ROW], DT, tag="wcat2")
            nc.sync.dma_start(
                out=wcat2_sb[:],
                in_=wcat2_t[:].rearrange("(t p) r -> p t r", p=P))
            xdT_sb = constp.tile([d.F, d.GRID], DT, tag="xdT")
            nc.sync.dma_start(out=xdT_sb[:], in_=xdT_t[:])
            bias1_sb = constp.tile([P, d.C], f32, tag="bias1")
            nc.sync.dma_start(out=bias1_sb[:], in_=bias1_t[:])
            bias2_sb = constp.tile([P, d.C], f32, tag="bias2")
            nc.sync.dma_start(out=bias2_sb[:], in_=bias2_t[:])
            iota_sb = constp.tile([P, P], f32, tag="iota")
            nc.sync.dma_start(out=iota_sb[:], in_=iota_t[:])
            zeros_sb = constp.tile([P, d.C], f32, tag="zeros")
            nc.vector.memset(zeros_sb[:], 0.0)
            ident = constp.tile([P, P], DT, tag="ident")
            make_identity(nc, ident[:])
            ident32 = constp.tile([P, P], f32, tag="ident32")
            make_identity(nc, ident32[:])
            gid_sb = constp.tile([P, d.NCH], f32, tag="gid")
            nc.sync.dma_start(out=gid_sb[:], in_=gid_t[:])
            pool_row_sb = constp.tile([P, 1], i32, tag="pool_row")
            nc.sync.dma_start(out=pool_row_sb[:], in_=pool_row_t[:])

            # ---- adg1: a_d per dst slot via PE ----
            adg1 = constp.tile([P, d.NCH, d.H], DT, tag="adg1")
            for g in range(d.NCH):
                adps = psC.tile([P, d.H], f32, tag="adps")
                nc.tensor.matmul(adps[:], lhsT=xdT_sb[:, g * P:(g + 1) * P],
                                 rhs=wad1_sb[:], start=True, stop=True)
                nc.scalar.activation(adg1[:, g, :], adps[:], AF.Copy)

            # ---- shared edge math ----
            def edge_math(gt, adg_g, ncols, msk, accn, accd, first):
                lg = small.tile([P, d.TMAX, d.H], f32, tag="lg")
                nc.vector.tensor_tensor(
                    out=lg[:, 0:ncols, :],
                    in0=gt[:, 0:ncols, d.C:d.C + d.H],
                    in1=adg_g.to_broadcast((P, ncols, d.H)),
                    op=OP.add)
                if msk is not None:
                    nc.vector.tensor_tensor(
                        out=lg[:, 0:ncols, :],
                        in0=lg[:, 0:ncols, :],
                        in1=msk[:, 0:ncols, :].to_broadcast((P, ncols, d.H)),
                        op=OP.add)
                nc.vector.scalar_tensor_tensor(
                    out=lg[:, 0:ncols, :], in0=lg[:, 0:ncols, :],
                    scalar=NEG_SLOPE, in1=lg[:, 0:ncols, :],
                    op0=OP.mult, op1=OP.max)
                wt = small.tile([P, d.TMAX, d.H], DT, tag="wt")
                nc.scalar.activation(wt[:, 0:ncols, :], lg[:, 0:ncols, :],
                                     AF.Exp)
                if first:
                    nout, dout = accn, accd
                else:
                    dout = small.tile([P, d.H], f32, tag="dpart")
                nc.vector.tensor_reduce(
                    out=dout[:],
                    in_=wt[:, 0:ncols, :].rearrange("p k h -> p h k"),
                    axis=mybir.AxisListType.X, op=OP.add)
                hview = gt[:, 0:ncols, 0:d.C].rearrange(
                    "p k (h ch) -> p k h ch", h=d.H)
                nc.vector.tensor_tensor(
                    out=hview, in0=hview,
                    in1=wt[:, 0:ncols, :].to_broadcast(
                        (P, ncols, d.H, d.CH)),
                    op=OP.mult)
                # contiguous fold-tree sum over k (in place, bf16)
                n = ncols
                while n > 1:
                    m = n // 2
                    nc.vector.tensor_add(gt[:, 0:m, 0:d.C],
                                         gt[:, 0:m, 0:d.C],
                                         gt[:, n - m:n, 0:d.C])
                    n -= m
                if first:
                    nc.vector.tensor_copy(nout[:], gt[:, 0, 0:d.C])
                else:
                    nc.vector.tensor_add(accn[:], accn[:], gt[:, 0, 0:d.C])
                    nc.vector.tensor_add(accd[:], accd[:], dout[:])

            def epilogue(accn, accd, bias_sb, out_tile):
                nc.vector.tensor_scalar_max(accd[:], accd[:], 1e-20)
                rcp = small.tile([P, d.H], f32, tag="rcp")
                nc.vector.reciprocal(rcp[:], accd[:])
                x_ = small.tile([P, d.C], f32, tag="x_")
                nc.vector.tensor_tensor(
                    out=x_[:].rearrange("p (h ch) -> p h ch", h=d.H),
                    in0=accn[:].rearrange("p (h ch) -> p h ch", h=d.H),
                    in1=rcp[:].to_broadcast((P, d.H, d.CH)),
                    op=OP.mult)
                nc.vector.tensor_add(x_[:], x_[:], bias_sb[:])
                ex = small.tile([P, d.C], f32, tag="ex")
                nc.scalar.activation(ex[:], x_[:], AF.Exp)
                nc.vector.scalar_tensor_tensor(
                    out=ex[:], in0=ex[:], scalar=-1.0, in1=zeros_sb[:],
                    op0=OP.add, op1=OP.min)
                nc.vector.tensor_scalar_max(x_[:], x_[:], 0.0)
                nc.vector.tensor_tensor(out=out_tile[:], in0=x_[:], in1=ex[:],
                                        op=OP.add)

            # ================= layer 1 (no gathers) =================
            accn = accd = None
            for (g, kst, ncols) in pl.pieces1:
                cs = int(pl.cs1[g]) + kst
                piece_first = kst == 0
                piece_last = kst + ncols == pl.K1[g]
                xsb = xp.tile([d.F, d.TMAX * P], DT, tag="xsb")
                nc.sync.dma_start(out=xsb[:, 0:ncols * P],
                                  in_=xeT_t[:, cs * P:(cs + ncols) * P])
                gt = gath.tile([P, d.TMAX, d.ROW], DT, tag="gt1")
                for kc in range(ncols):
                    ps = psA.tile([P, d.ROW], f32, tag="l1ps")
                    nc.tensor.matmul(ps[:],
                                     lhsT=xsb[:, kc * P:(kc + 1) * P],
                                     rhs=wcat1_sb[:], start=True, stop=True)
                    nc.scalar.activation(gt[:, kc, :], ps[:], AF.Copy)
                msk = small.tile([P, d.TMAX, 1], f32, tag="msk")
                nc.sync.dma_start(out=msk[:, 0:ncols, 0],
                                  in_=mask1_t[:, cs:cs + ncols])
                if piece_first:
                    accn = accp.tile([P, d.C], f32, tag="accn")
                    accd = accp.tile([P, d.H], f32, tag="accd")
                edge_math(gt, adg1[:, g:g + 1, :], ncols, msk, accn, accd,
                          piece_first)
                if piece_last:
                    et = work.tile([P, d.C], DT, tag="et1")
                    epilogue(accn, accd, bias1_sb, et)
                    elT = work.tile([P, CT, P], DT, tag="elT")
                    for it in range(CT):
                        tp = psB.tile([P, P], DT, tag="tp")
                        nc.tensor.transpose(tp[:], et[:, it * P:(it + 1) * P],
                                            ident[:])
                        nc.scalar.activation(elT[:, it, :], tp[:], AF.Copy)
                    ps2 = psA.tile([P, d.ROW], f32, tag="l1ps")
                    for it in range(CT):
                        nc.tensor.matmul(ps2[:], lhsT=elT[:, it, :],
                                         rhs=wcat2_sb[:, it, :],
                                         start=(it == 0), stop=(it == CT - 1))
                    h2t = work.tile([P, d.ROW], DT, tag="h2t")
                    nc.scalar.activation(h2t[:], ps2[:], AF.Copy)
                    nc.sync.dma_start(
                        out=htab2own[g * P:(g + 1) * P, 0:d.ROW], in_=h2t[:])

            # pad rows: h = 0, a_s/a_d = AS_PAD
            drow = work.tile([P, d.ROWG], DT, tag="drow")
            nc.vector.memset(drow[:, 0:d.C], 0.0)
            nc.vector.memset(drow[:, d.C:d.ROWG], AS_PAD)
            nc.sync.dma_start(out=htab2own[d.GRID:d.GRID_D, :], in_=drow[:])

            nc.gpsimd.collective_compute(
                "AllGather", OP.bypass,
                replica_groups=[list(range(d.n_cores))],
                ins=[htab2own.opt()], outs=[htab2all.opt()])

            # a_d table for layer 2 straight from own rows (same grid order)
            adg2 = constp.tile([P, d.NCH, d.H], DT, tag="adg2")
            nc.sync.dma_start(
                out=adg2[:],
                in_=htab2own[0:d.GRID, :].rearrange(
                    "(g p) r -> p g r", p=P)[:, :, d.C + d.H:d.C + 2 * d.H])

            # ================= layer 2 (single gathers) =================
            pool_ps = paccp.tile([P, d.C], f32, tag="poolps")
            lo_ap = htab2all[0:d.LO_ROWS, :]
            hi_ap = htab2all[d.LO_ROWS:NGRID_ALL, :]

            for (side, g, kst, ncols, cfirst, clast) in pl.pieces2:
                cs = int((pl.cslo if side == "lo" else pl.cshi)[g]) + kst
                off_t = off_lo_t if side == "lo" else off_hi_t
                nidx = P * ncols
                ita = small.tile([P, 8 * d.TMAX], i16, tag="ita")
                nc.sync.dma_start(out=ita[:, 0:8 * ncols],
                                  in_=off_t[:, 8 * cs:8 * (cs + ncols)])
                gt = gath.tile([P, d.TMAX, d.ROWG], DT, tag="gt2")
                nc.gpsimd.dma_gather(
                    out_ap=gt[:, 0:ncols, :],
                    in_ap=lo_ap if side == "lo" else hi_ap,
                    idxs_ap=ita[:, 0:8 * ncols],
                    num_idxs=nidx, num_idxs_reg=nidx,
                    elem_size=d.ROWG, single_packet=False)
                if cfirst:
                    accn = accp.tile([P, d.C], f32, tag="accn")
                    accd = accp.tile([P, d.H], f32, tag="accd")
                edge_math(gt, adg2[:, g:g + 1, :], ncols, None, accn, accd,
                          cfirst)
                if clast:
                    et2 = work.tile([P, d.C], f32, tag="et2")
                    epilogue(accn, accd, bias2_sb, et2)
                    oh = work.tile([P, P], f32, tag="oh")
                    nc.vector.tensor_tensor(
                        out=oh[:],
                        in0=gid_sb[:, g:g + 1].to_broadcast((P, P)),
                        in1=iota_sb[:], op=OP.is_equal)
                    nc.tensor.matmul(pool_ps[:], lhsT=oh[:], rhs=et2[:],
                                     start=(g == 0), stop=(g == d.NCH - 1))

            # ---- pool partial -> AllReduce -> mean -> fc ----
            zt = work.tile([P, d.C], f32, tag="zt")
            nc.vector.memset(zt[:], 0.0)
            for t in range(d.NGP // P):
                nc.sync.dma_start(out=poolpart[t * P:(t + 1) * P, :],
                                  in_=zt[:])
            pool_sb = work.tile([P, d.C], f32, tag="poolsb")
            nc.vector.tensor_copy(pool_sb[:], pool_ps[:])
            nc.gpsimd.indirect_dma_start(
                out=poolpart[:, :],
                out_offset=bass.IndirectOffsetOnAxis(ap=pool_row_sb[:, 0:1],
                                                     axis=0),
                in_=pool_sb[:], in_offset=None)
            nc.gpsimd.collective_compute(
                "AllReduce", OP.add,
                replica_groups=[list(range(d.n_cores))],
                ins=[poolpart.opt()], outs=[poolsum.opt()])

            rcp_sb = constp.tile([P, d.NG // P], f32, tag="rcp_cnt")
            nc.sync.dma_start(out=rcp_sb[:], in_=rcp_t[:])
            fcw_sb = constp.tile([P, CT, d.OUT], f32, tag="fcw")
            nc.sync.dma_start(
                out=fcw_sb[:],
                in_=fcw_t[:].rearrange("(t p) o -> p t o", p=P))
            fcb_sb = constp.tile([P, d.OUT], f32, tag="fcb")
            nc.sync.dma_start(out=fcb_sb[:], in_=fcb_t[:])
            for t in range(d.NG // P):
                pm = work.tile([P, d.C], f32, tag="pm")
                nc.sync.dma_start(out=pm[:],
                                  in_=poolsum[t * P:(t + 1) * P, :])
                nc.vector.tensor_scalar(
                    out=pm[:], in0=pm[:], scalar1=rcp_sb[:, t:t + 1],
                    scalar2=None, op0=OP.mult)
                pmT = work.tile([P, CT, P], f32, tag="pmT")
                for it in range(CT):
                    tp = psB.tile([P, P], f32, tag="tp")
                    nc.tensor.transpose(tp[:], pm[:, it * P:(it + 1) * P],
                                        ident32[:])
                    nc.vector.tensor_copy(pmT[:, it, :], tp[:])
                ops = psC.tile([P, d.OUT], f32, tag="ops")
                for it in range(CT):
                    nc.tensor.matmul(ops[:], lhsT=pmT[:, it, :],
                                     rhs=fcw_sb[:, it, :],
                                     start=(it == 0), stop=(it == CT - 1))
                ot = work.tile([P, d.OUT], f32, tag="ot")
                nc.vector.tensor_add(ot[:], ops[:], fcb_sb[:])
                nc.sync.dma_start(out=out_t[t * P:(t + 1) * P, :], in_=ot[:])

    nc.compile()
    return nc


def np_dt_of(table_dt):
    import ml_dtypes
    return {mybir.dt.bfloat16: ml_dtypes.bfloat16,
            mybir.dt.float32: np.float32}[table_dt]


TABLE_DT = mybir.dt.bfloat16

_CACHE = {}


def kernel(**inputs):
    """Full (unsharded) inputs -> full [512, 64] float32 output.

    Shards dst nodes/edges across 8 NeuronCores internally, compiles the
    Bass program for this graph, and runs it SPMD on cores 0-7.
    """
    from concourse.bass_utils import run_bass_kernel_spmd

    d = Dims()
    ei = np.asarray(inputs["edge_index"])
    bt = np.asarray(inputs["batch"])
    key = (ei.tobytes(), bt.tobytes())
    if key in _CACHE:
        pl, nc = _CACHE[key]
    else:
        pl = build_plan(ei, bt, d)
        nc = build_program(pl, TABLE_DT)
        _CACHE[key] = (pl, nc)
    in_maps = build_inputs(inputs, pl, np_dt_of(TABLE_DT))
    res = run_bass_kernel_spmd(nc, in_maps, core_ids=list(range(d.n_cores)))
    return np.asarray(res.results[0]["out"], dtype=np.float32)


if __name__ == "__main__":
    print("kernel.py v2")
